# revision 62
# baseline (speedup 1.0000x reference)
"""Trainium2 Bass kernel for nn_LRSVConv (low-rank spatially-varying conv).

Computes, for full inputs
    x            [8, 32, 256, 256]  f32
    conv_w       [192, 32, 3, 3]    f32   (192 = RANK(3) * C_OUT(64))
    kernel_weight[2, 256, 256]      f32
the reference:
    y   = conv2d(x, conv_w, stride 1, pad 1)      # [8, 192, 256, 256]
    y   = y.reshape(8, 3, 64, 256, 256)
    out = y[:,0] + kw[0]*y[:,1] + kw[1]*y[:,2]    # [8, 64, 256, 256]

Sharding: spatial (H) across 8 cores - each core computes a band of 32
output rows for all batches.

Per-core kernel design (v3, ~144 us on core 0 vs 367 us baseline):
  - K=128 packing: partitions = (kh' in 0..3, c_in), where replica kh'
    holds the band's input rows shifted by kh' (even local rows only -
    odd output rows read odd input rows from the odd-kh' replicas).
    M=128 = (e, c_out) with e the output-row parity within a row pair:
    stationary W[(kh',ci),(e,c)] = conv_w[64r+c, ci, kh'-e, kw]
    (zero outside 0<=kh'-e<=2).  One matmul thus covers 2 output rows x
    64 channels at full 128-wide array occupancy.
  - Supertile = 4 output rows (2 row pairs p2) x 256 cols -> N=512.
    9 conv matmuls per supertile (3 ranks x 3 kw): ranks 1,2 into one
    2-bank psum bc2 [128,(rk,p2,j)]; rank 0 into psum A [128,512].
    This is the streaming floor: 4.5 N-columns per output pixel given
    K<=128 / M<=128 / N<=512 (kw cannot fold into K next to the 4 kh'
    replicas, and 192 channels need 1.5 M-tiles).
  - Blend is partition-aligned (rows are (e,c) for every rank), no
    TensorE involvement so the conv stream never stalls:
      DVE:    m   = bc2 * sv12   (per-pixel weights, host-broadcast bf16)
      DVE:    s   = m[:,:512] + m[:,512:]   (bf16 SBUF 2x mode)
      Act:    a_bf = copy(A) -> bf16
      GpSimd: osb = a_bf + s     (only consumer is the out-DMA, so Q7
                                  slowness cannot backpressure the PE)
    retired RETIRE_DIST supertiles late; output DMA'd as bf16 (halves
    the write traffic; ~2e-3 extra rel err vs the 2e-2 budget).
  - Everything the PE consumes is bf16 (psum accumulation stays f32).
  - HAM management: the PE clock-gates to 1.2 GHz unless continuously
    busy ~3.4us.  WARMUP dummy N=64 matmuls bridge the head DMA wait so
    the real stream runs at 2.4 GHz from the first matmul (216 ns/MM).
  - Head latency: batch-0 imcol and the sv table are loaded as several
    separate tiles in first-use order (dependency tracking is
    tile-granular - one big tile would stall the first matmuls ~6us).
"""

import os

import ml_dtypes
import numpy as np

B, C_IN, C_OUT, RANK, IMG = 8, 32, 64, 3, 256
N_CORES = 8
BAND = IMG // N_CORES          # 32 output rows per core
WP = IMG + 2                   # padded width 258
NL = 16                        # even-local-row slots per partition
T = 8                          # supertiles per (batch, band): 4 rows each
NBLK = 512                     # matmul free size: (p2=2) x (j=256)

_F32 = np.float32
_BF16 = ml_dtypes.bfloat16

# "act": Act evacuates rank-0 psum to bf16 SBUF, DVE adds (bf16 2x mode).
# "pe":  fold s into rank-0 psum with an identity matmul, Act evacuates.
# "dve": out = A + s directly on DVE (psum operand, 1x mode).
BLEND = os.environ.get("KERNEL_BLEND", "act")
RETIRE_DIST = int(os.environ.get("KERNEL_RETIRE_DIST", "2"))
BC2_BUFS = int(os.environ.get("KERNEL_BC2_BUFS", "2"))
RETIRE_TOP = os.environ.get("KERNEL_RETIRE_TOP", "0") == "1"
ALT_FINAL = os.environ.get("KERNEL_ALT_FINAL", "0") == "1"
# dummy N=64 matmuls during the head DMA wait: keeps the PE busy through
# the HAM activity window so the first real matmuls run at 2.4 GHz
WARMUP = int(os.environ.get("KERNEL_WARMUP", "72"))


def _build_bass():
    import concourse.mybir as mybir
    import concourse.tile as tile
    from concourse import bacc

    f32 = mybir.dt.float32
    bf16 = mybir.dt.bfloat16
    nc = bacc.Bacc("TRN2", target_bir_lowering=False, debug=False)

    xs_t = nc.dram_tensor("xs", (B, 128, NL * WP), bf16, kind="ExternalInput")
    w_t = nc.dram_tensor("wconv", (RANK, 3, 128, 128), bf16, kind="ExternalInput")
    id_t = (
        nc.dram_tensor("ident", (128, 128), bf16, kind="ExternalInput")
        if BLEND == "pe"
        else None
    )
    sv_t = nc.dram_tensor("sv", (128, T * 2 * NBLK), bf16, kind="ExternalInput")
    out_t = nc.dram_tensor("out", (B, C_OUT, BAND, IMG), bf16, kind="ExternalOutput")

    xs = xs_t.ap()
    # band row = 4t + 2*p2 + e ; psum/sbuf rows are (e,c), cols (p2,j)
    out_r = out_t.ap().rearrange(
        "b c (t p2 e) w -> b e c t p2 w", t=T, p2=2, e=2
    )

    with tile.TileContext(nc) as tc:
        with (
            tc.tile_pool(name="const", bufs=1) as cpool,
            tc.tile_pool(name="imcol", bufs=2) as ipool,
            tc.tile_pool(name="psum", bufs=BC2_BUFS, space="PSUM") as ppool,
            tc.tile_pool(name="psacc", bufs=RETIRE_DIST + 1, space="PSUM") as papool,
            tc.tile_pool(name="mb", bufs=3) as mpool,
            tc.tile_pool(name="sb", bufs=RETIRE_DIST + 2) as spool,
            tc.tile_pool(name="ab", bufs=3) as apool,
            tc.tile_pool(name="outp", bufs=4) as opool,
        ):
            if WARMUP:
                dummy_w = cpool.tile([128, 64], bf16)
                nc.vector.memset(dummy_w[:], 0)
                # dummy psum borrows an acc slot; freed before the loop needs it
                dum_ps = papool.tile([64, 64], f32, tag="acc")
                for _ in range(WARMUP):
                    nc.tensor.matmul(
                        dum_ps[:], dummy_w[:], dummy_w[:, :64],
                        start=True, stop=True,
                    )
            # DMA order = dependency order of the first supertile: weights
            # (small) first, then batch 0's imcol in progressively larger
            # separate tiles (supertile t only depends on its own piece -
            # dep tracking is tile-granular), sv last (only needed by the
            # first m-op, ~2us later).
            w_sb = cpool.tile([128, RANK, 3, 128], bf16)
            nc.sync.dma_start(w_sb[:], w_t.ap().rearrange("r k p m -> p r k m"))
            im0_parts = []
            for lo, hi in ((0, 2), (2, 4), (4, 8), (8, NL)):
                part = ipool.tile([128, (hi - lo) * WP], bf16, tag=f"im0_{lo}")
                im0_parts.append([lo, hi, part])
            # sv split the same way (the m-op of supertile t must not wait on
            # the whole 2.1MB tile); DMAs interleaved in first-use order
            sv_parts = []
            for lo, hi in ((0, 1), (1, 3), (3, T)):
                part = cpool.tile(
                    [128, (hi - lo) * 2 * NBLK], bf16, tag=f"sv_{lo}"
                )
                sv_parts.append([lo, hi, part])
            nc.sync.dma_start(im0_parts[0][2][:], xs[0, :, 0 : 2 * WP])
            nc.sync.dma_start(im0_parts[1][2][:], xs[0, :, 2 * WP : 4 * WP])
            nc.sync.dma_start(sv_parts[0][2][:], sv_t.ap()[:, 0 : 2 * NBLK])
            nc.sync.dma_start(
                sv_parts[1][2][:], sv_t.ap()[:, 2 * NBLK : 6 * NBLK]
            )
            nc.sync.dma_start(im0_parts[2][2][:], xs[0, :, 4 * WP : 8 * WP])
            nc.sync.dma_start(im0_parts[3][2][:], xs[0, :, 8 * WP : NL * WP])
            nc.sync.dma_start(
                sv_parts[2][2][:], sv_t.ap()[:, 6 * NBLK : T * 2 * NBLK]
            )
            # batch 1's imcol issued ahead of batch 0's retire DMAs so it
            # doesn't queue behind ~14 output DMAs on the SP sequencer
            im1 = ipool.tile([128, NL * WP], bf16, tag="im")
            nc.sync.dma_start(im1[:], xs[1])

            def sv_for(t):
                for lo, hi, part in sv_parts:
                    if lo <= t < hi:
                        off = 2 * NBLK * (t - lo)
                        return part[:, off : off + 2 * NBLK]
                raise AssertionError
            if BLEND == "pe":
                id_sb = cpool.tile([128, 128], bf16)
                nc.sync.dma_start(id_sb[:], id_t.ap())

            def retire(pending, tail=False):
                ap_, s, pb, pt = pending
                osb = opool.tile([128, NBLK], bf16, tag="osb")
                if BLEND == "pe":
                    nc.tensor.matmul(ap_[:], id_sb[:], s[:], start=False, stop=True)
                    nc.scalar.copy(osb[:], ap_[:])
                elif BLEND == "act":
                    a_bf = apool.tile([128, NBLK], bf16, tag="a_bf")
                    nc.scalar.copy(a_bf[:], ap_[:])
                    # final add on GpSimd: its only consumer is the out-DMA,
                    # so Q7 slowness can't backpressure the PE pipeline.
                    # Drain-phase retires go on DVE (faster, nothing left to
                    # block).
                    eng = nc.vector if (ALT_FINAL or tail) else nc.gpsimd
                    eng.tensor_tensor(osb[:], a_bf[:], s[:], mybir.AluOpType.add)
                else:
                    nc.vector.tensor_tensor(
                        osb[:], ap_[:], s[:], mybir.AluOpType.add
                    )
                for e in range(2):
                    nc.sync.dma_start(
                        out_r[pb, e, :, pt], osb[64 * e : 64 * e + 64, :]
                    )

            def imv_for(b, im, t):
                """view [128, 2, WP+] for supertile t's two l-rows"""
                if b == 0:
                    for lo, hi, part in im0_parts:
                        if lo <= 2 * t < hi:
                            pv = part.rearrange("p (l w) -> p l w", w=WP)
                            return pv[:, 2 * t - lo : 2 * t - lo + 2, :]
                pv = im.rearrange("p (l w) -> p l w", w=WP)
                return pv[:, 2 * t : 2 * t + 2, :]

            pending = []
            for b in range(B):
                im = None
                if b == 1:
                    im = im1
                elif b > 1:
                    im = ipool.tile([128, NL * WP], bf16, tag="im")
                    nc.sync.dma_start(im[:], xs[b])
                for t in range(T):
                    if RETIRE_TOP and len(pending) >= RETIRE_DIST:
                        retire(pending.pop(0))
                    bc2 = ppool.tile([128, 2 * NBLK], f32, tag="bc2")
                    ap_ = papool.tile([128, NBLK], f32, tag="acc")
                    m = mpool.tile([128, 2 * NBLK], bf16, tag="m")
                    rhs = imv_for(b, im, t)
                    for r in (1, 2):
                        for kw in range(3):
                            nc.tensor.matmul(
                                bc2[:, NBLK * (r - 1) : NBLK * r],
                                w_sb[:, r, kw, :],
                                rhs[:, :, kw : kw + IMG],
                                start=(kw == 0),
                                stop=(kw == 2),
                            )
                    nc.vector.tensor_tensor(
                        m[:], bc2[:], sv_for(t), mybir.AluOpType.mult
                    )
                    for kw in range(3):
                        nc.tensor.matmul(
                            ap_[:],
                            w_sb[:, 0, kw, :],
                            rhs[:, :, kw : kw + IMG],
                            start=(kw == 0),
                            stop=(BLEND != "pe" and kw == 2),
                        )
                    # s on DVE right after m: bf16 SBUF 2x mode, no stall
                    # (m just finished on the same engine)
                    s = spool.tile([128, NBLK], bf16, tag="s")
                    nc.vector.tensor_add(s[:], m[:, 0:NBLK], m[:, NBLK : 2 * NBLK])
                    pending.append((ap_, s, b, t))
                    if not RETIRE_TOP and len(pending) > RETIRE_DIST:
                        retire(pending.pop(0))
            for p in pending:
                retire(p, tail=True)
    nc.compile()
    return nc


_CACHE = {}


def _get_bass():
    if "nc" not in _CACHE:
        _CACHE["nc"] = _build_bass()
    return _CACHE["nc"]


def _prep_shards(x, conv_w, kernel_weight):
    x = np.asarray(x, dtype=_F32)
    conv_w = np.asarray(conv_w, dtype=_F32)
    kernel_weight = np.asarray(kernel_weight, dtype=_F32)

    x_pad = np.pad(x, ((0, 0), (0, 0), (1, 1), (1, 1)))  # [B,32,258,258]

    # stationary: w[r, kw, (kh',ci), (e,c)] = conv_w[64r+c, ci, kh'-e, kw]
    cw5 = conv_w.reshape(RANK, C_OUT, C_IN, 3, 3)
    w = np.zeros((RANK, 3, 4, C_IN, 2, C_OUT), dtype=_F32)
    for e in range(2):
        for khp in range(4):
            kh = khp - e
            if 0 <= kh <= 2:
                # cw5[:, c, ci, kh, kw] -> (r, kw, ci, c)
                w[:, :, khp, :, e, :] = cw5[:, :, :, kh, :].transpose(0, 3, 2, 1)
    wfull = w.reshape(RANK, 3, 128, 128).astype(_BF16)

    ident = np.eye(128, dtype=_F32).astype(_BF16)

    in_maps = []
    for i in range(N_CORES):
        h0 = BAND * i
        # xs[b, 32*khp+ci, lr2, w] = x_pad[b, ci, h0 + 2*lr2 + khp, w]
        xband = x_pad[:, :, h0 : h0 + BAND + 2, :]  # [B,32,34,258]
        xsh = np.empty((B, 4, C_IN, NL, WP), dtype=_BF16)
        for khp in range(4):
            xsh[:, khp] = xband[:, :, khp : khp + 32 : 2, :]
        xsh = xsh.reshape(B, 128, NL * WP)

        # sv[(e,c), (t,rk,p2,j)] = kernel_weight[rk, h0+4t+2p2+e, j]
        kb = kernel_weight[:, h0 : h0 + BAND, :].reshape(2, T, 2, 2, IMG)
        svb = kb.transpose(3, 1, 0, 2, 4)  # [e, t, rk, p2, j]
        svb = np.broadcast_to(
            svb[:, None], (2, C_OUT, T, 2, 2, IMG)
        ).reshape(128, T * 2 * NBLK)
        svb = np.ascontiguousarray(svb).astype(_BF16)

        im = {"xs": xsh, "wconv": wfull, "sv": svb}
        if BLEND == "pe":
            im["ident"] = ident
        in_maps.append(im)
    return in_maps


def run(inputs, trace=False):
    """Run the sharded bass kernel; returns (out_full, BassKernelResults)."""
    from concourse.bass_utils import run_bass_kernel_spmd

    in_maps = _prep_shards(**inputs)
    nc = _get_bass()
    res = run_bass_kernel_spmd(
        nc, in_maps, core_ids=list(range(N_CORES)), trace=trace
    )
    out = np.empty((B, C_OUT, IMG, IMG), dtype=_F32)
    for i in range(N_CORES):
        out[:, :, BAND * i : BAND * (i + 1), :] = res.results[i]["out"].astype(_F32)
    return out, res


def kernel(x, conv_w, kernel_weight):
    out, _ = run({"x": x, "conv_w": conv_w, "kernel_weight": kernel_weight})
    return out


# revision 64
# speedup vs baseline: 1.1926x; 1.1926x over previous
"""Trainium2 Bass kernel for nn_LRSVConv (low-rank spatially-varying conv).

Computes, for full inputs
    x            [8, 32, 256, 256]  f32
    conv_w       [192, 32, 3, 3]    f32   (192 = RANK(3) * C_OUT(64))
    kernel_weight[2, 256, 256]      f32
the reference:
    y   = conv2d(x, conv_w, stride 1, pad 1)      # [8, 192, 256, 256]
    y   = y.reshape(8, 3, 64, 256, 256)
    out = y[:,0] + kw[0]*y[:,1] + kw[1]*y[:,2]    # [8, 64, 256, 256]

Sharding: spatial (H) across 8 cores - each core computes a band of 32
output rows for all batches.

Per-core kernel design (v3, ~144 us on core 0 vs 367 us baseline):
  - K=128 packing: partitions = (kh' in 0..3, c_in), where replica kh'
    holds the band's input rows shifted by kh' (even local rows only -
    odd output rows read odd input rows from the odd-kh' replicas).
    M=128 = (e, c_out) with e the output-row parity within a row pair:
    stationary W[(kh',ci),(e,c)] = conv_w[64r+c, ci, kh'-e, kw]
    (zero outside 0<=kh'-e<=2).  One matmul thus covers 2 output rows x
    64 channels at full 128-wide array occupancy.
  - Supertile = 4 output rows (2 row pairs p2) x 256 cols -> N=512.
    9 conv matmuls per supertile (3 ranks x 3 kw): ranks 1,2 into one
    2-bank psum bc2 [128,(rk,p2,j)]; rank 0 into psum A [128,512].
    This is the streaming floor: 4.5 N-columns per output pixel given
    K<=128 / M<=128 / N<=512 (kw cannot fold into K next to the 4 kh'
    replicas, and 192 channels need 1.5 M-tiles).
  - Blend is partition-aligned (rows are (e,c) for every rank), no
    TensorE involvement so the conv stream never stalls:
      DVE:    m   = bc2 * sv12   (per-pixel weights, host-broadcast bf16)
      DVE:    s   = m[:,:512] + m[:,512:]   (bf16 SBUF 2x mode)
      Act:    a_bf = copy(A) -> bf16
      GpSimd: osb = a_bf + s     (only consumer is the out-DMA, so Q7
                                  slowness cannot backpressure the PE)
    retired RETIRE_DIST supertiles late; output DMA'd as bf16 (halves
    the write traffic; ~2e-3 extra rel err vs the 2e-2 budget).
  - Everything the PE consumes is bf16 (psum accumulation stays f32).
  - HAM management: the PE clock-gates to 1.2 GHz unless continuously
    busy ~3.4us.  WARMUP dummy N=64 matmuls bridge the head DMA wait so
    the real stream runs at 2.4 GHz from the first matmul (216 ns/MM).
  - Head latency: batch-0 imcol and the sv table are loaded as several
    separate tiles in first-use order (dependency tracking is
    tile-granular - one big tile would stall the first matmuls ~6us).
"""

import os

import ml_dtypes
import numpy as np

B, C_IN, C_OUT, RANK, IMG = 8, 32, 64, 3, 256
N_CORES = 8
BAND = IMG // N_CORES          # 32 output rows per core
WP = IMG + 2                   # padded width 258
NL = 16                        # even-local-row slots per partition
T = 8                          # supertiles per (batch, band): 4 rows each
NBLK = 512                     # matmul free size: (p2=2) x (j=256)

_F32 = np.float32
_BF16 = ml_dtypes.bfloat16

# "act": Act evacuates rank-0 psum to bf16 SBUF, DVE adds (bf16 2x mode).
# "pe":  fold s into rank-0 psum with an identity matmul, Act evacuates.
# "dve": out = A + s directly on DVE (psum operand, 1x mode).
BLEND = os.environ.get("KERNEL_BLEND", "act")
RETIRE_DIST = int(os.environ.get("KERNEL_RETIRE_DIST", "2"))
BC2_BUFS = int(os.environ.get("KERNEL_BC2_BUFS", "2"))
RETIRE_TOP = os.environ.get("KERNEL_RETIRE_TOP", "0") == "1"
ALT_FINAL = os.environ.get("KERNEL_ALT_FINAL", "0") == "1"
# dummy N=64 matmuls during the head DMA wait: keeps the PE busy through
# the HAM activity window so the first real matmuls run at 2.4 GHz
WARMUP = int(os.environ.get("KERNEL_WARMUP", "72"))


def _build_bass():
    import concourse.mybir as mybir
    import concourse.tile as tile
    from concourse import bacc

    f32 = mybir.dt.float32
    bf16 = mybir.dt.bfloat16
    nc = bacc.Bacc("TRN2", target_bir_lowering=False, debug=False)

    xs_t = nc.dram_tensor("xs", (B, 128, NL * WP), bf16, kind="ExternalInput")
    w_t = nc.dram_tensor("wconv", (RANK, 3, 128, 128), bf16, kind="ExternalInput")
    id_t = (
        nc.dram_tensor("ident", (128, 128), bf16, kind="ExternalInput")
        if BLEND == "pe"
        else None
    )
    sv_t = nc.dram_tensor("sv", (128, T * 2 * NBLK), bf16, kind="ExternalInput")
    out_t = nc.dram_tensor("out", (B, C_OUT, BAND, IMG), bf16, kind="ExternalOutput")

    xs = xs_t.ap()
    # band row = 4t + 2*p2 + e ; psum/sbuf rows are (e,c), cols (p2,j)
    out_r = out_t.ap().rearrange(
        "b c (t p2 e) w -> b e c t p2 w", t=T, p2=2, e=2
    )

    with tile.TileContext(nc) as tc:
        with (
            tc.tile_pool(name="const", bufs=1) as cpool,
            tc.tile_pool(name="imcol", bufs=2) as ipool,
            tc.tile_pool(name="psum", bufs=BC2_BUFS, space="PSUM") as ppool,
            tc.tile_pool(name="psacc", bufs=RETIRE_DIST + 1, space="PSUM") as papool,
            tc.tile_pool(name="mb", bufs=3) as mpool,
            tc.tile_pool(name="sb", bufs=RETIRE_DIST + 2) as spool,
            tc.tile_pool(name="ab", bufs=3) as apool,
            tc.tile_pool(name="outp", bufs=4) as opool,
        ):
            if WARMUP:
                dummy_w = cpool.tile([128, 64], bf16)
                nc.vector.memset(dummy_w[:], 0)
                # dummy psum borrows an acc slot; freed before the loop needs it
                dum_ps = papool.tile([64, 64], f32, tag="acc")
                for _ in range(WARMUP):
                    nc.tensor.matmul(
                        dum_ps[:], dummy_w[:], dummy_w[:, :64],
                        start=True, stop=True,
                    )
            # DMA order = dependency order of the first supertile: weights
            # (small) first, then batch 0's imcol in progressively larger
            # separate tiles (supertile t only depends on its own piece -
            # dep tracking is tile-granular), sv last (only needed by the
            # first m-op, ~2us later).
            w_sb = cpool.tile([128, RANK, 3, 128], bf16)
            nc.sync.dma_start(w_sb[:], w_t.ap().rearrange("r k p m -> p r k m"))
            im0_parts = []
            for lo, hi in ((0, 2), (2, 4), (4, 8), (8, NL)):
                part = ipool.tile([128, (hi - lo) * WP], bf16, tag=f"im0_{lo}")
                im0_parts.append([lo, hi, part])
            # sv split the same way (the m-op of supertile t must not wait on
            # the whole 2.1MB tile); DMAs interleaved in first-use order
            sv_parts = []
            for lo, hi in ((0, 1), (1, 3), (3, T)):
                part = cpool.tile(
                    [128, (hi - lo) * 2 * NBLK], bf16, tag=f"sv_{lo}"
                )
                sv_parts.append([lo, hi, part])
            nc.sync.dma_start(im0_parts[0][2][:], xs[0, :, 0 : 2 * WP])
            nc.sync.dma_start(im0_parts[1][2][:], xs[0, :, 2 * WP : 4 * WP])
            nc.sync.dma_start(sv_parts[0][2][:], sv_t.ap()[:, 0 : 2 * NBLK])
            nc.sync.dma_start(
                sv_parts[1][2][:], sv_t.ap()[:, 2 * NBLK : 6 * NBLK]
            )
            nc.sync.dma_start(im0_parts[2][2][:], xs[0, :, 4 * WP : 8 * WP])
            nc.sync.dma_start(im0_parts[3][2][:], xs[0, :, 8 * WP : NL * WP])
            nc.sync.dma_start(
                sv_parts[2][2][:], sv_t.ap()[:, 6 * NBLK : T * 2 * NBLK]
            )

            def sv_for(t):
                for lo, hi, part in sv_parts:
                    if lo <= t < hi:
                        off = 2 * NBLK * (t - lo)
                        return part[:, off : off + 2 * NBLK]
                raise AssertionError
            if BLEND == "pe":
                id_sb = cpool.tile([128, 128], bf16)
                nc.sync.dma_start(id_sb[:], id_t.ap())

            def retire(pending, tail=False):
                ap_, s, pb, pt = pending
                osb = opool.tile([128, NBLK], bf16, tag="osb")
                if BLEND == "pe":
                    nc.tensor.matmul(ap_[:], id_sb[:], s[:], start=False, stop=True)
                    nc.scalar.copy(osb[:], ap_[:])
                elif BLEND == "act":
                    a_bf = apool.tile([128, NBLK], bf16, tag="a_bf")
                    nc.scalar.copy(a_bf[:], ap_[:])
                    # final add on GpSimd: its only consumer is the out-DMA,
                    # so Q7 slowness can't backpressure the PE pipeline.
                    # Drain-phase retires go on DVE (faster, nothing left to
                    # block).
                    eng = nc.vector if (ALT_FINAL or tail) else nc.gpsimd
                    eng.tensor_tensor(osb[:], a_bf[:], s[:], mybir.AluOpType.add)
                else:
                    nc.vector.tensor_tensor(
                        osb[:], ap_[:], s[:], mybir.AluOpType.add
                    )
                for e in range(2):
                    nc.sync.dma_start(
                        out_r[pb, e, :, pt], osb[64 * e : 64 * e + 64, :]
                    )

            def imv_for(b, im, t):
                """view [128, 2, WP+] for supertile t's two l-rows"""
                if b == 0:
                    for lo, hi, part in im0_parts:
                        if lo <= 2 * t < hi:
                            pv = part.rearrange("p (l w) -> p l w", w=WP)
                            return pv[:, 2 * t - lo : 2 * t - lo + 2, :]
                pv = im.rearrange("p (l w) -> p l w", w=WP)
                return pv[:, 2 * t : 2 * t + 2, :]

            pending = []
            for b in range(B):
                im = None
                if b > 0:
                    im = ipool.tile([128, NL * WP], bf16, tag="im")
                    nc.sync.dma_start(im[:], xs[b])
                for t in range(T):
                    if RETIRE_TOP and len(pending) >= RETIRE_DIST:
                        retire(pending.pop(0))
                    bc2 = ppool.tile([128, 2 * NBLK], f32, tag="bc2")
                    ap_ = papool.tile([128, NBLK], f32, tag="acc")
                    m = mpool.tile([128, 2 * NBLK], bf16, tag="m")
                    rhs = imv_for(b, im, t)
                    for r in (1, 2):
                        for kw in range(3):
                            nc.tensor.matmul(
                                bc2[:, NBLK * (r - 1) : NBLK * r],
                                w_sb[:, r, kw, :],
                                rhs[:, :, kw : kw + IMG],
                                start=(kw == 0),
                                stop=(kw == 2),
                            )
                    nc.vector.tensor_tensor(
                        m[:], bc2[:], sv_for(t), mybir.AluOpType.mult
                    )
                    for kw in range(3):
                        nc.tensor.matmul(
                            ap_[:],
                            w_sb[:, 0, kw, :],
                            rhs[:, :, kw : kw + IMG],
                            start=(kw == 0),
                            stop=(BLEND != "pe" and kw == 2),
                        )
                    # s on DVE right after m: bf16 SBUF 2x mode, no stall
                    # (m just finished on the same engine)
                    s = spool.tile([128, NBLK], bf16, tag="s")
                    nc.vector.tensor_add(s[:], m[:, 0:NBLK], m[:, NBLK : 2 * NBLK])
                    pending.append((ap_, s, b, t))
                    if not RETIRE_TOP and len(pending) > RETIRE_DIST:
                        retire(pending.pop(0))
            for p in pending:
                retire(p, tail=True)
    nc.compile()
    return nc


_CACHE = {}


def _get_bass():
    if "nc" not in _CACHE:
        _CACHE["nc"] = _build_bass()
    return _CACHE["nc"]


def _prep_shards(x, conv_w, kernel_weight):
    x = np.asarray(x, dtype=_F32)
    conv_w = np.asarray(conv_w, dtype=_F32)
    kernel_weight = np.asarray(kernel_weight, dtype=_F32)

    x_pad = np.pad(x, ((0, 0), (0, 0), (1, 1), (1, 1)))  # [B,32,258,258]

    # stationary: w[r, kw, (kh',ci), (e,c)] = conv_w[64r+c, ci, kh'-e, kw]
    cw5 = conv_w.reshape(RANK, C_OUT, C_IN, 3, 3)
    w = np.zeros((RANK, 3, 4, C_IN, 2, C_OUT), dtype=_F32)
    for e in range(2):
        for khp in range(4):
            kh = khp - e
            if 0 <= kh <= 2:
                # cw5[:, c, ci, kh, kw] -> (r, kw, ci, c)
                w[:, :, khp, :, e, :] = cw5[:, :, :, kh, :].transpose(0, 3, 2, 1)
    wfull = w.reshape(RANK, 3, 128, 128).astype(_BF16)

    ident = np.eye(128, dtype=_F32).astype(_BF16)

    in_maps = []
    for i in range(N_CORES):
        h0 = BAND * i
        # xs[b, 32*khp+ci, lr2, w] = x_pad[b, ci, h0 + 2*lr2 + khp, w]
        xband = x_pad[:, :, h0 : h0 + BAND + 2, :]  # [B,32,34,258]
        xsh = np.empty((B, 4, C_IN, NL, WP), dtype=_BF16)
        for khp in range(4):
            xsh[:, khp] = xband[:, :, khp : khp + 32 : 2, :]
        xsh = xsh.reshape(B, 128, NL * WP)

        # sv[(e,c), (t,rk,p2,j)] = kernel_weight[rk, h0+4t+2p2+e, j]
        kb = kernel_weight[:, h0 : h0 + BAND, :].reshape(2, T, 2, 2, IMG)
        svb = kb.transpose(3, 1, 0, 2, 4)  # [e, t, rk, p2, j]
        svb = np.broadcast_to(
            svb[:, None], (2, C_OUT, T, 2, 2, IMG)
        ).reshape(128, T * 2 * NBLK)
        svb = np.ascontiguousarray(svb).astype(_BF16)

        im = {"xs": xsh, "wconv": wfull, "sv": svb}
        if BLEND == "pe":
            im["ident"] = ident
        in_maps.append(im)
    return in_maps


def run(inputs, trace=False):
    """Run the sharded bass kernel; returns (out_full, BassKernelResults)."""
    from concourse.bass_utils import run_bass_kernel_spmd

    in_maps = _prep_shards(**inputs)
    nc = _get_bass()
    res = run_bass_kernel_spmd(
        nc, in_maps, core_ids=list(range(N_CORES)), trace=trace
    )
    out = np.empty((B, C_OUT, IMG, IMG), dtype=_F32)
    for i in range(N_CORES):
        out[:, :, BAND * i : BAND * (i + 1), :] = res.results[i]["out"].astype(_F32)
    return out, res


def kernel(x, conv_w, kernel_weight):
    out, _ = run({"x": x, "conv_w": conv_w, "kernel_weight": kernel_weight})
    return out
